# revision 50
# baseline (speedup 1.0000x reference)
"""Trainium2 Bass kernel for DescriptorMatcher (mutual nearest neighbor matching).

Problem: given desc0 [B,N,D], desc1 [B,M,D] (B=4, N=M=8192, D=128, fp32):
    sim     = desc0 @ desc1^T                      [B,N,M]
    score0  = max_m sim                            [B,N]
    match01 = argmax_m sim                         [B,N]
    match10 = argmax_n sim                         [B,M]
    valid   = (match10[match01[n]] == n) & (score0 > 0.1)
returns (match01, score0, valid).

Approach (consistent fp16-score world; the accuracy budget is a 2e-2
mismatch fraction, observed flips are ~90 of 32768 = 3e-3):

  Inputs are pre-rounded to fp16 on the host (halves input DMA; the DMA
  bandwidth pool is shared across all queues, so bytes are what matter).
  sim is computed with fp16 matmuls (PE: 1 cycle/row) accumulated in fp32
  PSUM and quantized to fp16 once in PSUM->SBUF copies on the ACT engine.
  Everything downstream -- row maxima, column maxima, the phase-2 equality
  search, and the mutual check -- operates on those exact fp16 values, so
  all reductions are exact maxima of a single well-defined matrix
  T = fp16(fp16(desc0) @ fp16(desc1)^T) and the mutual check
      match10[match01[n]] == n   <=>   score0[n] == colmax[match01[n]]
  holds exactly in the T-world (max is exact; fp16 rounding is monotone).
  Mismatches vs the fp32 reference only occur where argmax competitors sit
  within fp16-ulp / input-rounding noise of each other.

Sharding: 8 cores = 4 batches x 2 row-halves (4096 rows each).

Phase 1 (per core), for each of 32 n-tiles [128 rows x 8192 cols]:
    PE : 16 fp16 matmuls -> PSUM [128,2048] x4
    ACT: copy/cast PSUM -> SBUF fp16 row buffer [128, 8192]
    DVE: pairwise-max tree over [128,64,128] chunk view -> CM fp16 [128,64]
         (fp16 tensor_tensor runs 2x on DVE; tensor_reduce would not)
    DVE: colacc = max(colacc, row) fp16 (single full-width 2x op)
  the folded colacc [128, 8192] is DMA'd out per 2048-range as each range
  finalizes; the cheap 128->1 partition reduction happens on the host.
  Host: score0 = CM.max, c* = CM.argmax (128-wide chunk), group rows by c*.

Phase 2 (per core): for each group (rows sharing winning chunk c*, padded to
  128 slots = exactly one 128-row subtile), recompute sim[:, c*128:(c+1)*128]
  with a 128-wide fp16 matmul + identical ACT fp16 cast (bit-exact
  recompute; 8 subtiles share one [128,1024] PSUM tile and one wide copy),
  then max_index(score_fp16, chunk) gives the exact first-occurrence
  position of the row max. match01 = c*128 + within.

Rows overflowing a group's 128 slots (needs >128 of 4096 rows to share one
of 64 winning chunks, ~ +8 sigma) fall back to a host-side recompute.
"""

import numpy as np

import concourse.bass as bass  # noqa: F401  (bass must import before tile)
import concourse.mybir as mybir
import concourse.tile as tile
from concourse import bacc, bass_isa

B, N, M, D = 4, 8192, 8192, 128
NCORES = 8
HALF = N // 2          # rows per core
NT = HALF // 128       # 32 n-tiles per core
CW = 1024              # input-DMA chunk width
NCHUNK = 64            # 128-wide score chunks per row
CHW = M // NCHUNK      # 128
GCAP = 128             # phase-2 slots per chunk-group (1 subtile of 128;
                       # mean group 64, sigma 8 -> overflow ~ +8 sigma;
                       # overflow rows fall back to host, never wrong)
NST = NCHUNK * GCAP // 128   # 48 phase-2 sub-tiles
NSLOT = NCHUNK * GCAP        # 6144



def _build1():
    f32 = mybir.dt.float32
    f32r = mybir.dt.float32r
    f16 = mybir.dt.float16
    nc = bacc.Bacc("TRN2", target_bir_lowering=False, debug=False,
                   num_devices=NCORES)
    at = nc.dram_tensor("at", [D, HALF], f16, kind="ExternalInput").ap()
    bt = nc.dram_tensor("bt", [D, M], f16, kind="ExternalInput").ap()
    cm_o = nc.dram_tensor("cm", [128, NT * NCHUNK], f16,
                          kind="ExternalOutput").ap()
    colp_o = nc.dram_tensor("colp", [128, M], f16, kind="ExternalOutput").ap()

    with tile.TileContext(nc) as tc:
        with tc.tile_pool(name="big", bufs=1) as big, \
             tc.tile_pool(name="rows", bufs=3) as rows, \
             tc.tile_pool(name="scr", bufs=2) as scr, \
             tc.tile_pool(name="ps", bufs=2, space="PSUM") as ps:
            atb = big.tile([128, HALF], f16, name="atb")
            btb = big.tile([128, M], f16, name="btb")
            # spread input DMAs over the SP/ACT/gpsimd queues
            for c in range(0, HALF, CW):
                nc.gpsimd.dma_start(atb[:, c:c + CW], at[:, c:c + CW])
            for i, c in enumerate(range(0, M, CW)):
                eng = nc.sync if i % 2 == 0 else nc.scalar
                eng.dma_start(btb[:, c:c + CW], bt[:, c:c + CW])
            colacc = big.tile([128, M], f16, name="colacc")
            cm_all = big.tile([128, NT * NCHUNK], f16, name="cm_all")
            for t in range(NT):
                row = rows.tile([128, M], f16, tag="row", name="row")
                for c in range(4):
                    pt = ps.tile([128, 2048], f32, tag="pt", name="pt")
                    for j in range(4):
                        mlo = c * 2048 + j * 512
                        nc.tensor.matmul(pt[:, j * 512:(j + 1) * 512],
                                         atb[:, t * 128:(t + 1) * 128],
                                         btb[:, mlo:mlo + 512],
                                         start=True, stop=True)
                    nc.scalar.copy(row[:, c * 2048:(c + 1) * 2048], pt[:])
                # pairwise-max tree: [128,32,256] -> CM [128,32] (fp16 2x DVE)
                s = scr.tile([128, NCHUNK * 64], f16, tag="s", name="s")
                rv = row[:].rearrange("p (c w) -> p c w", w=CHW)
                sv = s[:].rearrange("p (c w) -> p c w", w=64)

                def tree(lo, hi):
                    nc.vector.tensor_tensor(sv[:, lo:hi, 0:64],
                                            rv[:, lo:hi, 0:64],
                                            rv[:, lo:hi, 64:128],
                                            op=mybir.AluOpType.max)
                    w = 32
                    while w >= 8:
                        nc.vector.tensor_tensor(sv[:, lo:hi, 0:w],
                                                sv[:, lo:hi, 0:w],
                                                sv[:, lo:hi, w:2 * w],
                                                op=mybir.AluOpType.max)
                        w //= 2
                    nc.vector.tensor_reduce(
                        cm_all[:, t * NCHUNK + lo:t * NCHUNK + hi],
                        sv[:, lo:hi, 0:8], axis=mybir.AxisListType.X,
                        op=mybir.AluOpType.max)

                # column-max fold (fp16 2x DVE; walrus has no Pool-engine max).
                # Last tile folds per 2048-range so the gpsimd partition
                # reduce of each finished range overlaps the remaining folds.
                if t == 0:
                    tree(0, NCHUNK)
                    nc.vector.tensor_copy(colacc[:], row[:])
                elif t < NT - 1:
                    tree(0, NCHUNK)
                    nc.vector.tensor_tensor(colacc[:], colacc[:], row[:],
                                            op=mybir.AluOpType.max)
                else:
                    tree(0, NCHUNK)
                    for c in range(4):
                        cs = slice(c * 2048, (c + 1) * 2048)
                        nc.vector.tensor_tensor(colacc[:, cs], colacc[:, cs],
                                                row[:, cs],
                                                op=mybir.AluOpType.max)
                        nc.sync.dma_start(colp_o[:, cs], colacc[:, cs])
            nc.sync.dma_start(cm_o[:], cm_all[:])
    nc.compile()
    return nc


def _build2():
    f32 = mybir.dt.float32
    f32r = mybir.dt.float32r
    f16 = mybir.dt.float16
    u32 = mybir.dt.uint32
    nc = bacc.Bacc("TRN2", target_bir_lowering=False, debug=False,
                   num_devices=NCORES)
    at2 = nc.dram_tensor("at2", [D, NSLOT], f16, kind="ExternalInput").ap()
    bt = nc.dram_tensor("bt", [D, M], f16, kind="ExternalInput").ap()
    sg = nc.dram_tensor("sg", [128, NST * 8], f16, kind="ExternalInput").ap()
    idx_o = nc.dram_tensor("idx", [128, NST * 8], u32,
                           kind="ExternalOutput").ap()
    with tile.TileContext(nc) as tc:
        with tc.tile_pool(name="big", bufs=1) as big, \
             tc.tile_pool(name="work", bufs=3) as work, \
             tc.tile_pool(name="ps", bufs=4, space="PSUM") as ps:
            a2b = big.tile([128, NSLOT], f16, name="a2b")
            btb = big.tile([128, M], f16, name="btb")
            sgb = big.tile([128, NST * 8], f16, name="sgb")
            nc.sync.dma_start(sgb[:], sg[:])
            # interleave per group-pair so group g's compute unlocks as soon
            # as its slots and columns land (DMA bandwidth is shared across
            # queues; ordering, not queue count, is what matters)
            for i in range(8):
                w8 = NSLOT // 8
                nc.scalar.dma_start(a2b[:, i * w8:(i + 1) * w8],
                                    at2[:, i * w8:(i + 1) * w8])
                eng = nc.sync if i % 2 == 0 else nc.gpsimd
                eng.dma_start(btb[:, i * CW:(i + 1) * CW],
                              bt[:, i * CW:(i + 1) * CW])
            idx8 = big.tile([128, NST * 8], u32, name="idx8")
            # 8 subtiles (one 128-wide chunk-group each) share one
            # [128,1024] PSUM tile and one contiguous ACT copy.
            for q in range(NST // 8):
                pt = ps.tile([128, 1024], f32, tag="pt", name="pt")
                for k in range(8):
                    st = q * 8 + k
                    nc.tensor.matmul(pt[:, k * CHW:(k + 1) * CHW],
                                     a2b[:, st * 128:(st + 1) * 128],
                                     btb[:, st * CHW:(st + 1) * CHW],
                                     start=True, stop=True)
                ch = work.tile([128, 8 * CHW], f16, tag="ch", name="ch")
                nc.scalar.copy(ch[:], pt[:])
                for k in range(8):
                    st = q * 8 + k
                    nc.vector.max_index(idx8[:, st * 8:(st + 1) * 8],
                                        sgb[:, st * 8:(st + 1) * 8],
                                        ch[:, k * CHW:(k + 1) * CHW])
            nc.sync.dma_start(idx_o[:], idx8[:])
    nc.compile()
    return nc


_cached = None


def _make_exec(nc):
    import jax
    from jax.sharding import Mesh, PartitionSpec
    from jax.experimental.shard_map import shard_map
    from concourse import bass2jax
    from concourse.bass2jax import _bass_exec_p

    partition_name = nc.partition_id_tensor.name if nc.partition_id_tensor else None
    in_names, out_names, out_avals, out_shapes = [], [], [], []
    for alloc in nc.m.functions[0].allocations:
        if not isinstance(alloc, mybir.MemoryLocationSet):
            continue
        name = alloc.memorylocations[0].name
        if alloc.kind == "ExternalInput":
            if name != partition_name:
                in_names.append(name)
        elif alloc.kind == "ExternalOutput":
            shape = tuple(alloc.tensor_shape)
            dtype = mybir.dt.np(alloc.dtype)
            out_names.append(name)
            out_shapes.append((shape, dtype))
            out_avals.append(jax.core.ShapedArray(shape, dtype))
    n_params = len(in_names)
    n_outs = len(out_names)
    all_in_names = in_names + out_names
    if partition_name is not None:
        all_in_names = all_in_names + [partition_name]

    def _body(*args):
        operands = list(args)
        if partition_name is not None:
            operands.append(bass2jax.partition_id_tensor())
        outs = _bass_exec_p.bind(
            *operands, out_avals=tuple(out_avals), in_names=tuple(all_in_names),
            out_names=tuple(out_names), lowering_input_output_aliases=(),
            sim_require_finite=True, sim_require_nnan=True, nc=nc)
        return tuple(outs)

    devices = jax.devices()[:NCORES]
    mesh = Mesh(np.asarray(devices), ("core",))
    in_specs = (PartitionSpec("core"),) * (n_params + n_outs)
    out_specs = (PartitionSpec("core"),) * n_outs
    fn = jax.jit(shard_map(_body, mesh=mesh, in_specs=in_specs,
                           out_specs=out_specs, check_rep=False),
                 keep_unused=True)
    return {"fn": fn, "in_names": in_names, "out_names": out_names,
            "out_shapes": out_shapes, "nc": nc}


def _run(ex, ins):
    """ins: dict name -> [NCORES, *shape]; returns dict name -> [NCORES, *shape]."""
    concat_in = [np.ascontiguousarray(ins[n].reshape(-1, *ins[n].shape[2:]))
                 for n in ex["in_names"]]
    concat_zeros = [np.zeros((NCORES * s[0], *s[1:]), dt)
                    for (s, dt) in ex["out_shapes"]]
    out_arrs = ex["fn"](*concat_in, *concat_zeros)
    return {name: np.asarray(out_arrs[i]).reshape(NCORES, *ex["out_shapes"][i][0])
            for i, name in enumerate(ex["out_names"])}


def kernel(desc0, desc1):
    global _cached
    desc0 = np.asarray(desc0, dtype=np.float32)
    desc1 = np.asarray(desc1, dtype=np.float32)
    assert desc0.shape == (B, N, D) and desc1.shape == (B, M, D)

    if _cached is None:
        _cached = (_make_exec(_build1()), _make_exec(_build2()))
    ex1, ex2 = _cached

    a_slab = np.stack([desc0[b, h * HALF:(h + 1) * HALF]
                       for b in range(B) for h in range(2)]) \
               .astype(np.float16)                                # [8,4096,128]
    bt_all = np.stack([desc1[b].transpose(1, 0)
                       for b in range(B) for h in range(2)]) \
               .astype(np.float16)                                # [8,128,8192]
    at_all = a_slab.transpose(0, 2, 1)                            # [8,128,4096]

    r1 = _run(ex1, {"at": at_all, "bt": bt_all})

    # host glue: score/chunk-argmax + grouping for phase 2 (all fp16-exact)
    cm = r1["cm"].reshape(NCORES, 128, NT, NCHUNK).transpose(0, 2, 1, 3) \
                 .reshape(NCORES, HALF, NCHUNK)
    score0_c = cm.max(axis=2)                                     # [8,4096] f16
    cstar_c = cm.argmax(axis=2)                                   # [8, 4096]

    at2 = np.zeros((NCORES, D, NSLOT), np.float16)
    sg = np.full((NCORES, 128, NST * 8), 60000.0, np.float16)
    slot_of_row = np.full((NCORES, HALF), -1, np.int64)
    overflow = []                                                 # (core, row)
    for core in range(NCORES):
        for g in range(NCHUNK):
            rows_g = np.nonzero(cstar_c[core] == g)[0]
            if len(rows_g) > GCAP:
                overflow.extend((core, r) for r in rows_g[GCAP:])
                rows_g = rows_g[:GCAP]
            slots = g * GCAP + np.arange(len(rows_g))
            slot_of_row[core, rows_g] = slots
            at2[core][:, slots] = a_slab[core][rows_g].T
            st, lane = slots // 128, slots % 128
            for k in range(8):
                sg[core][lane, st * 8 + k] = score0_c[core][rows_g]

    r2 = _run(ex2, {"at2": at2, "bt": bt_all, "sg": sg})
    within = r2["idx"][:, :, ::8]                                 # [8, 128, NST]

    match01 = np.empty((B, N), dtype=np.int32)
    score0 = np.empty((B, N), dtype=np.float32)
    valid = np.empty((B, N), dtype=bool)
    # 128->1 partition reduction of the per-core column maxima on the host
    colmax = r1["colp"].astype(np.float32) \
                       .reshape(B, 2 * 128, M).max(axis=1)        # [B, M]

    for core in range(NCORES):
        b, h = divmod(core, 2)
        s = score0_c[core]
        sl = slot_of_row[core]
        m = cstar_c[core] * CHW + \
            within[core][sl % 128, sl // 128].astype(np.int64)
        sel = slice(h * HALF, (h + 1) * HALF)
        score0[b, sel] = s.astype(np.float32)
        match01[b, sel] = m.astype(np.int32)
        valid[b, sel] = (s > 0.1) & (s == colmax[b][m])

    for core, row in overflow:                                    # ~never taken
        b, h = divmod(core, 2)
        simrow = a_slab[core][row].astype(np.float32) @ desc1[b].T
        n = h * HALF + row
        match01[b, n] = int(simrow.argmax())
        score0[b, n] = simrow.max()
        valid[b, n] = (score0[b, n] > 0.1) & \
                      (np.float16(score0[b, n]) == colmax[b][match01[b, n]])

    return match01, score0, valid


# revision 51
# speedup vs baseline: 1.0591x; 1.0591x over previous
"""Trainium2 Bass kernel for DescriptorMatcher (mutual nearest neighbor matching).

Problem: given desc0 [B,N,D], desc1 [B,M,D] (B=4, N=M=8192, D=128, fp32):
    sim     = desc0 @ desc1^T                      [B,N,M]
    score0  = max_m sim                            [B,N]
    match01 = argmax_m sim                         [B,N]
    match10 = argmax_n sim                         [B,M]
    valid   = (match10[match01[n]] == n) & (score0 > 0.1)
returns (match01, score0, valid).

Approach (consistent fp16-score world; the accuracy budget is a 2e-2
mismatch fraction, observed flips are ~90 of 32768 = 3e-3):

  Inputs are pre-rounded to fp16 on the host (halves input DMA; the DMA
  bandwidth pool is shared across all queues, so bytes are what matter).
  sim is computed with fp16 matmuls (PE: 1 cycle/row) accumulated in fp32
  PSUM and quantized to fp16 once in PSUM->SBUF copies on the ACT engine.
  Everything downstream -- row maxima, column maxima, the phase-2 equality
  search, and the mutual check -- operates on those exact fp16 values, so
  all reductions are exact maxima of a single well-defined matrix
  T = fp16(fp16(desc0) @ fp16(desc1)^T) and the mutual check
      match10[match01[n]] == n   <=>   score0[n] == colmax[match01[n]]
  holds exactly in the T-world (max is exact; fp16 rounding is monotone).
  Mismatches vs the fp32 reference only occur where argmax competitors sit
  within fp16-ulp / input-rounding noise of each other.

Sharding: 8 cores = 4 batches x 2 row-halves (4096 rows each).

Phase 1 (per core), for each of 32 n-tiles [128 rows x 8192 cols]:
    PE : 16 fp16 matmuls -> PSUM [128,2048] x4
    ACT: copy/cast PSUM -> SBUF fp16 row buffer [128, 8192]
    DVE: pairwise-max tree over [128,64,128] chunk view -> CM fp16 [128,64]
         (fp16 tensor_tensor runs 2x on DVE; tensor_reduce would not)
    DVE: colacc = max(colacc, row) fp16 (single full-width 2x op)
  the folded colacc [128, 8192] is DMA'd out per 2048-range as each range
  finalizes; the cheap 128->1 partition reduction happens on the host.
  Host: score0 = CM.max, c* = CM.argmax (128-wide chunk), group rows by c*.

Phase 2 (per core): for each group (rows sharing winning chunk c*, padded to
  128 slots = exactly one 128-row subtile), recompute sim[:, c*128:(c+1)*128]
  with a 128-wide fp16 matmul + identical ACT fp16 cast (bit-exact
  recompute; 8 subtiles share one [128,1024] PSUM tile and one wide copy),
  then max_index(score_fp16, chunk) gives the exact first-occurrence
  position of the row max. match01 = c*128 + within.

Rows overflowing a group's 128 slots (needs >128 of 4096 rows to share one
of 64 winning chunks, ~ +8 sigma) fall back to a host-side recompute.
"""

import numpy as np

import concourse.bass as bass  # noqa: F401  (bass must import before tile)
import concourse.mybir as mybir
import concourse.tile as tile
from concourse import bacc, bass_isa

B, N, M, D = 4, 8192, 8192, 128
NCORES = 8
HALF = N // 2          # rows per core
NT = HALF // 128       # 32 n-tiles per core
CW = 1024              # input-DMA chunk width
NCHUNK = 64            # 128-wide score chunks per row
CHW = M // NCHUNK      # 128
GCAP = 128             # phase-2 slots per chunk-group (1 subtile of 128;
                       # mean group 64, sigma 8 -> overflow ~ +8 sigma;
                       # overflow rows fall back to host, never wrong)
NST = NCHUNK * GCAP // 128   # 48 phase-2 sub-tiles
NSLOT = NCHUNK * GCAP        # 6144



def _build1():
    f32 = mybir.dt.float32
    f32r = mybir.dt.float32r
    f16 = mybir.dt.float16
    nc = bacc.Bacc("TRN2", target_bir_lowering=False, debug=False,
                   num_devices=NCORES)
    at = nc.dram_tensor("at", [D, HALF], f16, kind="ExternalInput").ap()
    bt = nc.dram_tensor("bt", [D, M], f16, kind="ExternalInput").ap()
    cm_o = nc.dram_tensor("cm", [128, NT * NCHUNK * 8], f16,
                          kind="ExternalOutput").ap()
    colp_o = nc.dram_tensor("colp", [128, M], f16, kind="ExternalOutput").ap()

    with tile.TileContext(nc) as tc:
        with tc.tile_pool(name="big", bufs=1) as big, \
             tc.tile_pool(name="rows", bufs=3) as rows, \
             tc.tile_pool(name="scr", bufs=2) as scr, \
             tc.tile_pool(name="ps", bufs=2, space="PSUM") as ps:
            atb = big.tile([128, HALF], f16, name="atb")
            btb = big.tile([128, M], f16, name="btb")
            # spread input DMAs over the SP/ACT/gpsimd queues
            for c in range(0, HALF, CW):
                nc.gpsimd.dma_start(atb[:, c:c + CW], at[:, c:c + CW])
            for i, c in enumerate(range(0, M, CW)):
                eng = nc.sync if i % 2 == 0 else nc.scalar
                eng.dma_start(btb[:, c:c + CW], bt[:, c:c + CW])
            colacc = big.tile([128, M], f16, name="colacc")
            cm_all = big.tile([128, NT * NCHUNK * 8], f16, name="cm_all")
            for t in range(NT):
                row = rows.tile([128, M], f16, tag="row", name="row")
                for c in range(4):
                    pt = ps.tile([128, 2048], f32, tag="pt", name="pt")
                    for j in range(4):
                        mlo = c * 2048 + j * 512
                        nc.tensor.matmul(pt[:, j * 512:(j + 1) * 512],
                                         atb[:, t * 128:(t + 1) * 128],
                                         btb[:, mlo:mlo + 512],
                                         start=True, stop=True)
                    nc.scalar.copy(row[:, c * 2048:(c + 1) * 2048], pt[:])
                # pairwise-max tree: [128,32,256] -> CM [128,32] (fp16 2x DVE)
                s = scr.tile([128, NCHUNK * 64], f16, tag="s", name="s")
                rv = row[:].rearrange("p (c w) -> p c w", w=CHW)
                sv = s[:].rearrange("p (c w) -> p c w", w=64)

                def tree(lo, hi):
                    # stops at 8 survivors per chunk, written straight into
                    # the cm slab; the host takes the final 8->1 max (saves
                    # the no-2x treduce and one tree level on the DVE)
                    nc.vector.tensor_tensor(sv[:, lo:hi, 0:64],
                                            rv[:, lo:hi, 0:64],
                                            rv[:, lo:hi, 64:128],
                                            op=mybir.AluOpType.max)
                    for w in (32, 16):
                        nc.vector.tensor_tensor(sv[:, lo:hi, 0:w],
                                                sv[:, lo:hi, 0:w],
                                                sv[:, lo:hi, w:2 * w],
                                                op=mybir.AluOpType.max)
                    cmv = cm_all[:, t * NCHUNK * 8 + lo * 8:
                                 t * NCHUNK * 8 + hi * 8] \
                        .rearrange("p (c w) -> p c w", w=8)
                    nc.vector.tensor_tensor(cmv[:, :, :], sv[:, lo:hi, 0:8],
                                            sv[:, lo:hi, 8:16],
                                            op=mybir.AluOpType.max)
                    nc.sync.dma_start(
                        cm_o[:, t * NCHUNK * 8:(t + 1) * NCHUNK * 8],
                        cm_all[:, t * NCHUNK * 8:(t + 1) * NCHUNK * 8])

                # column-max fold (fp16 2x DVE; walrus has no Pool-engine max).
                # Last tile folds per 2048-range so the gpsimd partition
                # reduce of each finished range overlaps the remaining folds.
                if t == 0:
                    tree(0, NCHUNK)
                    nc.vector.tensor_copy(colacc[:], row[:])
                elif t < NT - 1:
                    tree(0, NCHUNK)
                    nc.vector.tensor_tensor(colacc[:], colacc[:], row[:],
                                            op=mybir.AluOpType.max)
                else:
                    tree(0, NCHUNK)
                    for c in range(4):
                        cs = slice(c * 2048, (c + 1) * 2048)
                        nc.vector.tensor_tensor(colacc[:, cs], colacc[:, cs],
                                                row[:, cs],
                                                op=mybir.AluOpType.max)
                        nc.sync.dma_start(colp_o[:, cs], colacc[:, cs])
    nc.compile()
    return nc


def _build2():
    f32 = mybir.dt.float32
    f32r = mybir.dt.float32r
    f16 = mybir.dt.float16
    u32 = mybir.dt.uint32
    nc = bacc.Bacc("TRN2", target_bir_lowering=False, debug=False,
                   num_devices=NCORES)
    at2 = nc.dram_tensor("at2", [D, NSLOT], f16, kind="ExternalInput").ap()
    bt = nc.dram_tensor("bt", [D, M], f16, kind="ExternalInput").ap()
    sg = nc.dram_tensor("sg", [128, NST * 8], f16, kind="ExternalInput").ap()
    idx_o = nc.dram_tensor("idx", [128, NST * 8], u32,
                           kind="ExternalOutput").ap()
    with tile.TileContext(nc) as tc:
        with tc.tile_pool(name="big", bufs=1) as big, \
             tc.tile_pool(name="work", bufs=3) as work, \
             tc.tile_pool(name="ps", bufs=4, space="PSUM") as ps:
            a2b = big.tile([128, NSLOT], f16, name="a2b")
            btb = big.tile([128, M], f16, name="btb")
            sgb = big.tile([128, NST * 8], f16, name="sgb")
            nc.sync.dma_start(sgb[:], sg[:])
            # interleave per group-pair so group g's compute unlocks as soon
            # as its slots and columns land (DMA bandwidth is shared across
            # queues; ordering, not queue count, is what matters)
            for i in range(8):
                w8 = NSLOT // 8
                nc.scalar.dma_start(a2b[:, i * w8:(i + 1) * w8],
                                    at2[:, i * w8:(i + 1) * w8])
                eng = nc.sync if i % 2 == 0 else nc.gpsimd
                eng.dma_start(btb[:, i * CW:(i + 1) * CW],
                              bt[:, i * CW:(i + 1) * CW])
            idx8 = big.tile([128, NST * 8], u32, name="idx8")
            # 8 subtiles (one 128-wide chunk-group each) share one
            # [128,1024] PSUM tile and one contiguous ACT copy.
            for q in range(NST // 8):
                pt = ps.tile([128, 1024], f32, tag="pt", name="pt")
                for k in range(8):
                    st = q * 8 + k
                    nc.tensor.matmul(pt[:, k * CHW:(k + 1) * CHW],
                                     a2b[:, st * 128:(st + 1) * 128],
                                     btb[:, st * CHW:(st + 1) * CHW],
                                     start=True, stop=True)
                ch = work.tile([128, 8 * CHW], f16, tag="ch", name="ch")
                nc.scalar.copy(ch[:], pt[:])
                for k in range(8):
                    st = q * 8 + k
                    nc.vector.max_index(idx8[:, st * 8:(st + 1) * 8],
                                        sgb[:, st * 8:(st + 1) * 8],
                                        ch[:, k * CHW:(k + 1) * CHW])
            nc.sync.dma_start(idx_o[:], idx8[:])
    nc.compile()
    return nc


_cached = None


def _make_exec(nc):
    import jax
    from jax.sharding import Mesh, PartitionSpec
    from jax.experimental.shard_map import shard_map
    from concourse import bass2jax
    from concourse.bass2jax import _bass_exec_p

    partition_name = nc.partition_id_tensor.name if nc.partition_id_tensor else None
    in_names, out_names, out_avals, out_shapes = [], [], [], []
    for alloc in nc.m.functions[0].allocations:
        if not isinstance(alloc, mybir.MemoryLocationSet):
            continue
        name = alloc.memorylocations[0].name
        if alloc.kind == "ExternalInput":
            if name != partition_name:
                in_names.append(name)
        elif alloc.kind == "ExternalOutput":
            shape = tuple(alloc.tensor_shape)
            dtype = mybir.dt.np(alloc.dtype)
            out_names.append(name)
            out_shapes.append((shape, dtype))
            out_avals.append(jax.core.ShapedArray(shape, dtype))
    n_params = len(in_names)
    n_outs = len(out_names)
    all_in_names = in_names + out_names
    if partition_name is not None:
        all_in_names = all_in_names + [partition_name]

    def _body(*args):
        operands = list(args)
        if partition_name is not None:
            operands.append(bass2jax.partition_id_tensor())
        outs = _bass_exec_p.bind(
            *operands, out_avals=tuple(out_avals), in_names=tuple(all_in_names),
            out_names=tuple(out_names), lowering_input_output_aliases=(),
            sim_require_finite=True, sim_require_nnan=True, nc=nc)
        return tuple(outs)

    devices = jax.devices()[:NCORES]
    mesh = Mesh(np.asarray(devices), ("core",))
    in_specs = (PartitionSpec("core"),) * (n_params + n_outs)
    out_specs = (PartitionSpec("core"),) * n_outs
    fn = jax.jit(shard_map(_body, mesh=mesh, in_specs=in_specs,
                           out_specs=out_specs, check_rep=False),
                 keep_unused=True)
    return {"fn": fn, "in_names": in_names, "out_names": out_names,
            "out_shapes": out_shapes, "nc": nc}


def _run(ex, ins):
    """ins: dict name -> [NCORES, *shape]; returns dict name -> [NCORES, *shape]."""
    concat_in = [np.ascontiguousarray(ins[n].reshape(-1, *ins[n].shape[2:]))
                 for n in ex["in_names"]]
    concat_zeros = [np.zeros((NCORES * s[0], *s[1:]), dt)
                    for (s, dt) in ex["out_shapes"]]
    out_arrs = ex["fn"](*concat_in, *concat_zeros)
    return {name: np.asarray(out_arrs[i]).reshape(NCORES, *ex["out_shapes"][i][0])
            for i, name in enumerate(ex["out_names"])}


def kernel(desc0, desc1):
    global _cached
    desc0 = np.asarray(desc0, dtype=np.float32)
    desc1 = np.asarray(desc1, dtype=np.float32)
    assert desc0.shape == (B, N, D) and desc1.shape == (B, M, D)

    if _cached is None:
        _cached = (_make_exec(_build1()), _make_exec(_build2()))
    ex1, ex2 = _cached

    a_slab = np.stack([desc0[b, h * HALF:(h + 1) * HALF]
                       for b in range(B) for h in range(2)]) \
               .astype(np.float16)                                # [8,4096,128]
    bt_all = np.stack([desc1[b].transpose(1, 0)
                       for b in range(B) for h in range(2)]) \
               .astype(np.float16)                                # [8,128,8192]
    at_all = a_slab.transpose(0, 2, 1)                            # [8,128,4096]

    r1 = _run(ex1, {"at": at_all, "bt": bt_all})

    # host glue: score/chunk-argmax + grouping for phase 2 (all fp16-exact)
    cm = r1["cm"].reshape(NCORES, 128, NT, NCHUNK, 8).max(axis=4) \
                 .transpose(0, 2, 1, 3).reshape(NCORES, HALF, NCHUNK)
    score0_c = cm.max(axis=2)                                     # [8,4096] f16
    cstar_c = cm.argmax(axis=2)                                   # [8, 4096]

    at2 = np.zeros((NCORES, D, NSLOT), np.float16)
    sg = np.full((NCORES, 128, NST * 8), 60000.0, np.float16)
    slot_of_row = np.full((NCORES, HALF), -1, np.int64)
    overflow = []                                                 # (core, row)
    for core in range(NCORES):
        for g in range(NCHUNK):
            rows_g = np.nonzero(cstar_c[core] == g)[0]
            if len(rows_g) > GCAP:
                overflow.extend((core, r) for r in rows_g[GCAP:])
                rows_g = rows_g[:GCAP]
            slots = g * GCAP + np.arange(len(rows_g))
            slot_of_row[core, rows_g] = slots
            at2[core][:, slots] = a_slab[core][rows_g].T
            st, lane = slots // 128, slots % 128
            for k in range(8):
                sg[core][lane, st * 8 + k] = score0_c[core][rows_g]

    r2 = _run(ex2, {"at2": at2, "bt": bt_all, "sg": sg})
    within = r2["idx"][:, :, ::8]                                 # [8, 128, NST]

    match01 = np.empty((B, N), dtype=np.int32)
    score0 = np.empty((B, N), dtype=np.float32)
    valid = np.empty((B, N), dtype=bool)
    # 128->1 partition reduction of the per-core column maxima on the host
    colmax = r1["colp"].astype(np.float32) \
                       .reshape(B, 2 * 128, M).max(axis=1)        # [B, M]

    for core in range(NCORES):
        b, h = divmod(core, 2)
        s = score0_c[core]
        sl = slot_of_row[core]
        m = cstar_c[core] * CHW + \
            within[core][sl % 128, sl // 128].astype(np.int64)
        sel = slice(h * HALF, (h + 1) * HALF)
        score0[b, sel] = s.astype(np.float32)
        match01[b, sel] = m.astype(np.int32)
        valid[b, sel] = (s > 0.1) & (s == colmax[b][m])

    for core, row in overflow:                                    # ~never taken
        b, h = divmod(core, 2)
        simrow = a_slab[core][row].astype(np.float32) @ desc1[b].T
        n = h * HALF + row
        match01[b, n] = int(simrow.argmax())
        score0[b, n] = simrow.max()
        valid[b, n] = (score0[b, n] > 0.1) & \
                      (np.float16(score0[b, n]) == colmax[b][match01[b, n]])

    return match01, score0, valid


# revision 52
# speedup vs baseline: 1.1723x; 1.1069x over previous
"""Trainium2 Bass kernel for DescriptorMatcher (mutual nearest neighbor matching).

Problem: given desc0 [B,N,D], desc1 [B,M,D] (B=4, N=M=8192, D=128, fp32):
    sim     = desc0 @ desc1^T                      [B,N,M]
    score0  = max_m sim                            [B,N]
    match01 = argmax_m sim                         [B,N]
    match10 = argmax_n sim                         [B,M]
    valid   = (match10[match01[n]] == n) & (score0 > 0.1)
returns (match01, score0, valid).

Approach (consistent fp16-score world; the accuracy budget is a 2e-2
mismatch fraction, observed flips are ~90 of 32768 = 3e-3):

  Inputs are pre-rounded to fp16 on the host (halves input DMA; the DMA
  bandwidth pool is shared across all queues, so bytes are what matter).
  sim is computed with fp16 matmuls (PE: 1 cycle/row) accumulated in fp32
  PSUM and quantized to fp16 once in PSUM->SBUF copies on the ACT engine.
  Everything downstream -- row maxima, column maxima, the phase-2 equality
  search, and the mutual check -- operates on those exact fp16 values, so
  all reductions are exact maxima of a single well-defined matrix
  T = fp16(fp16(desc0) @ fp16(desc1)^T) and the mutual check
      match10[match01[n]] == n   <=>   score0[n] == colmax[match01[n]]
  holds exactly in the T-world (max is exact; fp16 rounding is monotone).
  Mismatches vs the fp32 reference only occur where argmax competitors sit
  within fp16-ulp / input-rounding noise of each other.

Sharding: 8 cores = 4 batches x 2 row-halves (4096 rows each).

Phase 1 (per core), for each of 32 n-tiles [128 rows x 8192 cols]:
    PE : 16 fp16 matmuls -> PSUM [128,2048] x4
    ACT: copy/cast PSUM -> SBUF fp16 row buffer [128, 8192]
    DVE: pairwise-max tree over [128,64,128] chunk view -> CM fp16 [128,64]
         (fp16 tensor_tensor runs 2x on DVE; tensor_reduce would not)
    DVE: colacc = max(colacc, row) fp16 (single full-width 2x op)
  the folded colacc [128, 8192] is DMA'd out per 2048-range as each range
  finalizes; the cheap 128->1 partition reduction happens on the host.
  Host: score0 = CM.max, c* = CM.argmax (128-wide chunk), group rows by c*.

Phase 2 (per core): for each group (rows sharing winning chunk c*, padded to
  128 slots = exactly one 128-row subtile), recompute sim[:, c*128:(c+1)*128]
  with a 128-wide fp16 matmul + identical ACT fp16 cast (bit-exact
  recompute; 8 subtiles share one [128,1024] PSUM tile and one wide copy),
  then max_index(score_fp16, chunk) gives the exact first-occurrence
  position of the row max. match01 = c*128 + within.

Rows overflowing a group's 128 slots (needs >128 of 4096 rows to share one
of 64 winning chunks, ~ +8 sigma) fall back to a host-side recompute.
"""

import numpy as np

import concourse.bass as bass  # noqa: F401  (bass must import before tile)
import concourse.mybir as mybir
import concourse.tile as tile
from concourse import bacc, bass_isa

B, N, M, D = 4, 8192, 8192, 128
NCORES = 8
HALF = N // 2          # rows per core
NT = HALF // 128       # 32 n-tiles per core
CW = 1024              # input-DMA chunk width
NCHUNK = 64            # 128-wide score chunks per row
CHW = M // NCHUNK      # 128
GCAP = 128             # phase-2 slots per chunk-group (1 subtile of 128;
                       # mean group 64, sigma 8 -> overflow ~ +8 sigma;
                       # overflow rows fall back to host, never wrong)
NST = NCHUNK * GCAP // 128   # 48 phase-2 sub-tiles
NSLOT = NCHUNK * GCAP        # 6144



def _build1():
    f32 = mybir.dt.float32
    f32r = mybir.dt.float32r
    f16 = mybir.dt.float16
    nc = bacc.Bacc("TRN2", target_bir_lowering=False, debug=False,
                   num_devices=NCORES)
    at = nc.dram_tensor("at", [D, HALF], f16, kind="ExternalInput").ap()
    bt = nc.dram_tensor("bt", [D, M], f16, kind="ExternalInput").ap()
    cm_o = nc.dram_tensor("cm", [128, NT * NCHUNK * 64], f16,
                          kind="ExternalOutput").ap()
    colp_o = nc.dram_tensor("colp", [128, M], f16, kind="ExternalOutput").ap()

    with tile.TileContext(nc) as tc:
        with tc.tile_pool(name="big", bufs=1) as big, \
             tc.tile_pool(name="rows", bufs=3) as rows, \
             tc.tile_pool(name="scr", bufs=2) as scr, \
             tc.tile_pool(name="ps", bufs=2, space="PSUM") as ps:
            atb = big.tile([128, HALF], f16, name="atb")
            btb = big.tile([128, M], f16, name="btb")
            # spread input DMAs over the SP/ACT/gpsimd queues
            for c in range(0, HALF, CW):
                nc.gpsimd.dma_start(atb[:, c:c + CW], at[:, c:c + CW])
            for i, c in enumerate(range(0, M, CW)):
                eng = nc.sync if i % 2 == 0 else nc.scalar
                eng.dma_start(btb[:, c:c + CW], bt[:, c:c + CW])
            colacc = big.tile([128, M], f16, name="colacc")
            for t in range(NT):
                row = rows.tile([128, M], f16, tag="row", name="row")
                for c in range(4):
                    pt = ps.tile([128, 2048], f32, tag="pt", name="pt")
                    for j in range(4):
                        mlo = c * 2048 + j * 512
                        nc.tensor.matmul(pt[:, j * 512:(j + 1) * 512],
                                         atb[:, t * 128:(t + 1) * 128],
                                         btb[:, mlo:mlo + 512],
                                         start=True, stop=True)
                    nc.scalar.copy(row[:, c * 2048:(c + 1) * 2048], pt[:])
                # single pairwise-max level: 64 survivors per 128-chunk,
                # streamed to the host which takes the per-chunk max (the
                # deeper tree levels cost more DVE than the extra DMA+host)
                s = scr.tile([128, NCHUNK * 64], f16, tag="s", name="s")
                rv = row[:].rearrange("p (c w) -> p c w", w=CHW)
                sv = s[:].rearrange("p (c w) -> p c w", w=64)

                def tree(lo, hi):
                    nc.vector.tensor_tensor(sv[:, lo:hi, :],
                                            rv[:, lo:hi, 0:64],
                                            rv[:, lo:hi, 64:128],
                                            op=mybir.AluOpType.max)
                    nc.sync.dma_start(
                        cm_o[:, t * NCHUNK * 64:(t + 1) * NCHUNK * 64],
                        s[:])

                # column-max fold (fp16 2x DVE; walrus has no Pool-engine max).
                # Last tile folds per 2048-range so the gpsimd partition
                # reduce of each finished range overlaps the remaining folds.
                if t == 0:
                    tree(0, NCHUNK)
                    nc.vector.tensor_copy(colacc[:], row[:])
                elif t < NT - 1:
                    tree(0, NCHUNK)
                    nc.vector.tensor_tensor(colacc[:], colacc[:], row[:],
                                            op=mybir.AluOpType.max)
                else:
                    tree(0, NCHUNK)
                    for c in range(4):
                        cs = slice(c * 2048, (c + 1) * 2048)
                        nc.vector.tensor_tensor(colacc[:, cs], colacc[:, cs],
                                                row[:, cs],
                                                op=mybir.AluOpType.max)
                        nc.sync.dma_start(colp_o[:, cs], colacc[:, cs])
    nc.compile()
    return nc


def _build2():
    f32 = mybir.dt.float32
    f32r = mybir.dt.float32r
    f16 = mybir.dt.float16
    u32 = mybir.dt.uint32
    nc = bacc.Bacc("TRN2", target_bir_lowering=False, debug=False,
                   num_devices=NCORES)
    at2 = nc.dram_tensor("at2", [D, NSLOT], f16, kind="ExternalInput").ap()
    bt = nc.dram_tensor("bt", [D, M], f16, kind="ExternalInput").ap()
    sg = nc.dram_tensor("sg", [128, NST * 8], f16, kind="ExternalInput").ap()
    idx_o = nc.dram_tensor("idx", [128, NST * 8], u32,
                           kind="ExternalOutput").ap()
    with tile.TileContext(nc) as tc:
        with tc.tile_pool(name="big", bufs=1) as big, \
             tc.tile_pool(name="work", bufs=3) as work, \
             tc.tile_pool(name="ps", bufs=4, space="PSUM") as ps:
            a2b = big.tile([128, NSLOT], f16, name="a2b")
            btb = big.tile([128, M], f16, name="btb")
            sgb = big.tile([128, NST * 8], f16, name="sgb")
            nc.sync.dma_start(sgb[:], sg[:])
            # interleave per group-pair so group g's compute unlocks as soon
            # as its slots and columns land (DMA bandwidth is shared across
            # queues; ordering, not queue count, is what matters)
            for i in range(8):
                w8 = NSLOT // 8
                nc.scalar.dma_start(a2b[:, i * w8:(i + 1) * w8],
                                    at2[:, i * w8:(i + 1) * w8])
                eng = nc.sync if i % 2 == 0 else nc.gpsimd
                eng.dma_start(btb[:, i * CW:(i + 1) * CW],
                              bt[:, i * CW:(i + 1) * CW])
            idx8 = big.tile([128, NST * 8], u32, name="idx8")
            # 8 subtiles (one 128-wide chunk-group each) share one
            # [128,1024] PSUM tile and one contiguous ACT copy.
            for q in range(NST // 8):
                pt = ps.tile([128, 1024], f32, tag="pt", name="pt")
                for k in range(8):
                    st = q * 8 + k
                    nc.tensor.matmul(pt[:, k * CHW:(k + 1) * CHW],
                                     a2b[:, st * 128:(st + 1) * 128],
                                     btb[:, st * CHW:(st + 1) * CHW],
                                     start=True, stop=True)
                ch = work.tile([128, 8 * CHW], f16, tag="ch", name="ch")
                nc.scalar.copy(ch[:], pt[:])
                for k in range(8):
                    st = q * 8 + k
                    nc.vector.max_index(idx8[:, st * 8:(st + 1) * 8],
                                        sgb[:, st * 8:(st + 1) * 8],
                                        ch[:, k * CHW:(k + 1) * CHW])
            nc.sync.dma_start(idx_o[:], idx8[:])
    nc.compile()
    return nc


_cached = None


def _make_exec(nc):
    import jax
    from jax.sharding import Mesh, PartitionSpec
    from jax.experimental.shard_map import shard_map
    from concourse import bass2jax
    from concourse.bass2jax import _bass_exec_p

    partition_name = nc.partition_id_tensor.name if nc.partition_id_tensor else None
    in_names, out_names, out_avals, out_shapes = [], [], [], []
    for alloc in nc.m.functions[0].allocations:
        if not isinstance(alloc, mybir.MemoryLocationSet):
            continue
        name = alloc.memorylocations[0].name
        if alloc.kind == "ExternalInput":
            if name != partition_name:
                in_names.append(name)
        elif alloc.kind == "ExternalOutput":
            shape = tuple(alloc.tensor_shape)
            dtype = mybir.dt.np(alloc.dtype)
            out_names.append(name)
            out_shapes.append((shape, dtype))
            out_avals.append(jax.core.ShapedArray(shape, dtype))
    n_params = len(in_names)
    n_outs = len(out_names)
    all_in_names = in_names + out_names
    if partition_name is not None:
        all_in_names = all_in_names + [partition_name]

    def _body(*args):
        operands = list(args)
        if partition_name is not None:
            operands.append(bass2jax.partition_id_tensor())
        outs = _bass_exec_p.bind(
            *operands, out_avals=tuple(out_avals), in_names=tuple(all_in_names),
            out_names=tuple(out_names), lowering_input_output_aliases=(),
            sim_require_finite=True, sim_require_nnan=True, nc=nc)
        return tuple(outs)

    devices = jax.devices()[:NCORES]
    mesh = Mesh(np.asarray(devices), ("core",))
    in_specs = (PartitionSpec("core"),) * (n_params + n_outs)
    out_specs = (PartitionSpec("core"),) * n_outs
    fn = jax.jit(shard_map(_body, mesh=mesh, in_specs=in_specs,
                           out_specs=out_specs, check_rep=False),
                 keep_unused=True)
    return {"fn": fn, "in_names": in_names, "out_names": out_names,
            "out_shapes": out_shapes, "nc": nc}


def _run(ex, ins):
    """ins: dict name -> [NCORES, *shape]; returns dict name -> [NCORES, *shape]."""
    concat_in = [np.ascontiguousarray(ins[n].reshape(-1, *ins[n].shape[2:]))
                 for n in ex["in_names"]]
    concat_zeros = [np.zeros((NCORES * s[0], *s[1:]), dt)
                    for (s, dt) in ex["out_shapes"]]
    out_arrs = ex["fn"](*concat_in, *concat_zeros)
    return {name: np.asarray(out_arrs[i]).reshape(NCORES, *ex["out_shapes"][i][0])
            for i, name in enumerate(ex["out_names"])}


def kernel(desc0, desc1):
    global _cached
    desc0 = np.asarray(desc0, dtype=np.float32)
    desc1 = np.asarray(desc1, dtype=np.float32)
    assert desc0.shape == (B, N, D) and desc1.shape == (B, M, D)

    if _cached is None:
        _cached = (_make_exec(_build1()), _make_exec(_build2()))
    ex1, ex2 = _cached

    a_slab = np.stack([desc0[b, h * HALF:(h + 1) * HALF]
                       for b in range(B) for h in range(2)]) \
               .astype(np.float16)                                # [8,4096,128]
    bt_all = np.stack([desc1[b].transpose(1, 0)
                       for b in range(B) for h in range(2)]) \
               .astype(np.float16)                                # [8,128,8192]
    at_all = a_slab.transpose(0, 2, 1)                            # [8,128,4096]

    r1 = _run(ex1, {"at": at_all, "bt": bt_all})

    # host glue: score/chunk-argmax + grouping for phase 2 (all fp16-exact)
    cm8 = r1["cm"].reshape(NCORES, 128, NT, NCHUNK, 64)
    cm = np.empty((NCORES, 128, NT, NCHUNK), np.float32)
    for core in range(NCORES):
        cm[core] = cm8[core].astype(np.float32).max(axis=3)
    cm = cm.transpose(0, 2, 1, 3).reshape(NCORES, HALF, NCHUNK)
    score0_c = cm.max(axis=2)                                     # [8,4096] f16
    cstar_c = cm.argmax(axis=2)                                   # [8, 4096]

    at2 = np.zeros((NCORES, D, NSLOT), np.float16)
    sg = np.full((NCORES, 128, NST * 8), 60000.0, np.float16)
    slot_of_row = np.full((NCORES, HALF), -1, np.int64)
    overflow = []                                                 # (core, row)
    for core in range(NCORES):
        for g in range(NCHUNK):
            rows_g = np.nonzero(cstar_c[core] == g)[0]
            if len(rows_g) > GCAP:
                overflow.extend((core, r) for r in rows_g[GCAP:])
                rows_g = rows_g[:GCAP]
            slots = g * GCAP + np.arange(len(rows_g))
            slot_of_row[core, rows_g] = slots
            at2[core][:, slots] = a_slab[core][rows_g].T
            st, lane = slots // 128, slots % 128
            for k in range(8):
                sg[core][lane, st * 8 + k] = score0_c[core][rows_g]

    r2 = _run(ex2, {"at2": at2, "bt": bt_all, "sg": sg})
    within = r2["idx"][:, :, ::8]                                 # [8, 128, NST]

    match01 = np.empty((B, N), dtype=np.int32)
    score0 = np.empty((B, N), dtype=np.float32)
    valid = np.empty((B, N), dtype=bool)
    # 128->1 partition reduction of the per-core column maxima on the host
    colmax = r1["colp"].astype(np.float32) \
                       .reshape(B, 2 * 128, M).max(axis=1)        # [B, M]

    for core in range(NCORES):
        b, h = divmod(core, 2)
        s = score0_c[core]
        sl = slot_of_row[core]
        m = cstar_c[core] * CHW + \
            within[core][sl % 128, sl // 128].astype(np.int64)
        sel = slice(h * HALF, (h + 1) * HALF)
        score0[b, sel] = s.astype(np.float32)
        match01[b, sel] = m.astype(np.int32)
        valid[b, sel] = (s > 0.1) & (s == colmax[b][m])

    for core, row in overflow:                                    # ~never taken
        b, h = divmod(core, 2)
        simrow = a_slab[core][row].astype(np.float32) @ desc1[b].T
        n = h * HALF + row
        match01[b, n] = int(simrow.argmax())
        score0[b, n] = simrow.max()
        valid[b, n] = (score0[b, n] > 0.1) & \
                      (np.float16(score0[b, n]) == colmax[b][match01[b, n]])

    return match01, score0, valid


# revision 53
# speedup vs baseline: 1.1872x; 1.0127x over previous
"""Trainium2 Bass kernel for DescriptorMatcher (mutual nearest neighbor matching).

Problem: given desc0 [B,N,D], desc1 [B,M,D] (B=4, N=M=8192, D=128, fp32):
    sim     = desc0 @ desc1^T                      [B,N,M]
    score0  = max_m sim                            [B,N]
    match01 = argmax_m sim                         [B,N]
    match10 = argmax_n sim                         [B,M]
    valid   = (match10[match01[n]] == n) & (score0 > 0.1)
returns (match01, score0, valid).

Approach (consistent fp16-score world; the accuracy budget is a 2e-2
mismatch fraction, observed flips are ~90 of 32768 = 3e-3):

  Inputs are pre-rounded to fp16 on the host (halves input DMA; the DMA
  bandwidth pool is shared across all queues, so bytes are what matter).
  sim is computed with fp16 matmuls (PE: 1 cycle/row) accumulated in fp32
  PSUM and quantized to fp16 once in PSUM->SBUF copies on the ACT engine.
  Everything downstream -- row maxima, column maxima, the phase-2 equality
  search, and the mutual check -- operates on those exact fp16 values, so
  all reductions are exact maxima of a single well-defined matrix
  T = fp16(fp16(desc0) @ fp16(desc1)^T) and the mutual check
      match10[match01[n]] == n   <=>   score0[n] == colmax[match01[n]]
  holds exactly in the T-world (max is exact; fp16 rounding is monotone).
  Mismatches vs the fp32 reference only occur where argmax competitors sit
  within fp16-ulp / input-rounding noise of each other.

Sharding: 8 cores = 4 batches x 2 row-halves (4096 rows each).

Phase 1 (per core), for each of 32 n-tiles [128 rows x 8192 cols]:
    PE : 16 fp16 matmuls -> PSUM [128,2048] x4
    ACT: copy/cast PSUM -> SBUF fp16 row buffer [128, 8192]
    DVE: pairwise-max tree over [128,64,128] chunk view -> CM fp16 [128,64]
         (fp16 tensor_tensor runs 2x on DVE; tensor_reduce would not)
    DVE: colacc = max(colacc, row) fp16 (single full-width 2x op)
  the folded colacc [128, 8192] is DMA'd out per 2048-range as each range
  finalizes; the cheap 128->1 partition reduction happens on the host.
  Host: score0 = CM.max, c* = CM.argmax (128-wide chunk), group rows by c*.

Phase 2 (per core): for each group (rows sharing winning chunk c*, padded to
  128 slots = exactly one 128-row subtile), recompute sim[:, c*128:(c+1)*128]
  with a 128-wide fp16 matmul + identical ACT fp16 cast (bit-exact
  recompute; 8 subtiles share one [128,1024] PSUM tile and one wide copy),
  then max_index(score_fp16, chunk) gives the exact first-occurrence
  position of the row max. match01 = c*128 + within.

Rows overflowing a group's 128 slots (needs >128 of 4096 rows to share one
of 64 winning chunks, ~ +8 sigma) fall back to a host-side recompute.
"""

import numpy as np

import concourse.bass as bass  # noqa: F401  (bass must import before tile)
import concourse.mybir as mybir
import concourse.tile as tile
from concourse import bacc, bass_isa

B, N, M, D = 4, 8192, 8192, 128
NCORES = 8
HALF = N // 2          # rows per core
NT = HALF // 128       # 32 n-tiles per core
CW = 1024              # input-DMA chunk width
NCHUNK = 64            # 128-wide score chunks per row
CHW = M // NCHUNK      # 128
GCAP = 128             # phase-2 slots per chunk-group (1 subtile of 128;
                       # mean group 64, sigma 8 -> overflow ~ +8 sigma;
                       # overflow rows fall back to host, never wrong)
NST = NCHUNK * GCAP // 128   # 48 phase-2 sub-tiles
NSLOT = NCHUNK * GCAP        # 6144



def _build1():
    f32 = mybir.dt.float32
    f32r = mybir.dt.float32r
    f16 = mybir.dt.float16
    nc = bacc.Bacc("TRN2", target_bir_lowering=False, debug=False,
                   num_devices=NCORES)
    at = nc.dram_tensor("at", [D, HALF], f16, kind="ExternalInput").ap()
    bt = nc.dram_tensor("bt", [D, M], f16, kind="ExternalInput").ap()
    cm_o = nc.dram_tensor("cm", [128, NT * NCHUNK * 64], f16,
                          kind="ExternalOutput").ap()
    colp_o = nc.dram_tensor("colp", [128, M], f16, kind="ExternalOutput").ap()

    with tile.TileContext(nc) as tc:
        with tc.tile_pool(name="big", bufs=1) as big, \
             tc.tile_pool(name="rows", bufs=3) as rows, \
             tc.tile_pool(name="scr", bufs=2) as scr, \
             tc.tile_pool(name="ps", bufs=2, space="PSUM") as ps:
            atb = big.tile([128, HALF], f16, name="atb")
            btb = big.tile([128, M], f16, name="btb")
            # spread input DMAs over the SP/ACT/gpsimd queues
            for c in range(0, HALF, CW):
                nc.gpsimd.dma_start(atb[:, c:c + CW], at[:, c:c + CW])
            for i, c in enumerate(range(0, M, CW)):
                eng = nc.sync if i % 2 == 0 else nc.scalar
                eng.dma_start(btb[:, c:c + CW], bt[:, c:c + CW])
            colacc = big.tile([128, M], f16, name="colacc")
            for t in range(NT):
                row = rows.tile([128, M], f16, tag="row", name="row")
                for c in range(4):
                    pt = ps.tile([128, 2048], f32, tag="pt", name="pt")
                    for j in range(4):
                        mlo = c * 2048 + j * 512
                        nc.tensor.matmul(pt[:, j * 512:(j + 1) * 512],
                                         atb[:, t * 128:(t + 1) * 128],
                                         btb[:, mlo:mlo + 512],
                                         start=True, stop=True)
                    if c == 0:
                        # ACT is the copy bottleneck; DVE has slack, so it
                        # casts columns [0:768] (phase 2 must cast those
                        # columns on DVE as well for bit-exact equality)
                        nc.vector.tensor_copy(row[:, 0:768], pt[:, 0:768])
                        nc.scalar.copy(row[:, 768:2048], pt[:, 768:2048])
                    else:
                        nc.scalar.copy(row[:, c * 2048:(c + 1) * 2048], pt[:])
                # single pairwise-max level: 64 survivors per 128-chunk,
                # streamed to the host which takes the per-chunk max (the
                # deeper tree levels cost more DVE than the extra DMA+host)
                s = scr.tile([128, NCHUNK * 64], f16, tag="s", name="s")
                rv = row[:].rearrange("p (c w) -> p c w", w=CHW)
                sv = s[:].rearrange("p (c w) -> p c w", w=64)

                def tree(lo, hi):
                    nc.vector.tensor_tensor(sv[:, lo:hi, :],
                                            rv[:, lo:hi, 0:64],
                                            rv[:, lo:hi, 64:128],
                                            op=mybir.AluOpType.max)
                    nc.sync.dma_start(
                        cm_o[:, t * NCHUNK * 64:(t + 1) * NCHUNK * 64],
                        s[:])

                # column-max fold (fp16 2x DVE; walrus has no Pool-engine max).
                # Last tile folds per 2048-range so the gpsimd partition
                # reduce of each finished range overlaps the remaining folds.
                if t == 0:
                    tree(0, NCHUNK)
                    nc.vector.tensor_copy(colacc[:], row[:])
                elif t < NT - 1:
                    tree(0, NCHUNK)
                    nc.vector.tensor_tensor(colacc[:], colacc[:], row[:],
                                            op=mybir.AluOpType.max)
                else:
                    tree(0, NCHUNK)
                    for c in range(4):
                        cs = slice(c * 2048, (c + 1) * 2048)
                        nc.vector.tensor_tensor(colacc[:, cs], colacc[:, cs],
                                                row[:, cs],
                                                op=mybir.AluOpType.max)
                        nc.sync.dma_start(colp_o[:, cs], colacc[:, cs])
    nc.compile()
    return nc


def _build2():
    f32 = mybir.dt.float32
    f32r = mybir.dt.float32r
    f16 = mybir.dt.float16
    u32 = mybir.dt.uint32
    nc = bacc.Bacc("TRN2", target_bir_lowering=False, debug=False,
                   num_devices=NCORES)
    at2 = nc.dram_tensor("at2", [D, NSLOT], f16, kind="ExternalInput").ap()
    bt = nc.dram_tensor("bt", [D, M], f16, kind="ExternalInput").ap()
    sg = nc.dram_tensor("sg", [128, NST * 8], f16, kind="ExternalInput").ap()
    idx_o = nc.dram_tensor("idx", [128, NST * 8], u32,
                           kind="ExternalOutput").ap()
    with tile.TileContext(nc) as tc:
        with tc.tile_pool(name="big", bufs=1) as big, \
             tc.tile_pool(name="work", bufs=3) as work, \
             tc.tile_pool(name="ps", bufs=4, space="PSUM") as ps:
            a2b = big.tile([128, NSLOT], f16, name="a2b")
            btb = big.tile([128, M], f16, name="btb")
            sgb = big.tile([128, NST * 8], f16, name="sgb")
            nc.sync.dma_start(sgb[:], sg[:])
            # interleave per group-pair so group g's compute unlocks as soon
            # as its slots and columns land (DMA bandwidth is shared across
            # queues; ordering, not queue count, is what matters)
            for i in range(8):
                w8 = NSLOT // 8
                nc.scalar.dma_start(a2b[:, i * w8:(i + 1) * w8],
                                    at2[:, i * w8:(i + 1) * w8])
                eng = nc.sync if i % 2 == 0 else nc.gpsimd
                eng.dma_start(btb[:, i * CW:(i + 1) * CW],
                              bt[:, i * CW:(i + 1) * CW])
            idx8 = big.tile([128, NST * 8], u32, name="idx8")
            # 8 subtiles (one 128-wide chunk-group each) share one
            # [128,1024] PSUM tile and one contiguous ACT copy.
            for q in range(NST // 8):
                pt = ps.tile([128, 1024], f32, tag="pt", name="pt")
                for k in range(8):
                    st = q * 8 + k
                    nc.tensor.matmul(pt[:, k * CHW:(k + 1) * CHW],
                                     a2b[:, st * 128:(st + 1) * 128],
                                     btb[:, st * CHW:(st + 1) * CHW],
                                     start=True, stop=True)
                ch = work.tile([128, 8 * CHW], f16, tag="ch", name="ch")
                if q == 0:
                    nc.vector.tensor_copy(ch[:, 0:768], pt[:, 0:768])
                    nc.scalar.copy(ch[:, 768:1024], pt[:, 768:1024])
                else:
                    nc.scalar.copy(ch[:], pt[:])
                for k in range(8):
                    st = q * 8 + k
                    nc.vector.max_index(idx8[:, st * 8:(st + 1) * 8],
                                        sgb[:, st * 8:(st + 1) * 8],
                                        ch[:, k * CHW:(k + 1) * CHW])
            nc.sync.dma_start(idx_o[:], idx8[:])
    nc.compile()
    return nc


_cached = None


def _make_exec(nc):
    import jax
    from jax.sharding import Mesh, PartitionSpec
    from jax.experimental.shard_map import shard_map
    from concourse import bass2jax
    from concourse.bass2jax import _bass_exec_p

    partition_name = nc.partition_id_tensor.name if nc.partition_id_tensor else None
    in_names, out_names, out_avals, out_shapes = [], [], [], []
    for alloc in nc.m.functions[0].allocations:
        if not isinstance(alloc, mybir.MemoryLocationSet):
            continue
        name = alloc.memorylocations[0].name
        if alloc.kind == "ExternalInput":
            if name != partition_name:
                in_names.append(name)
        elif alloc.kind == "ExternalOutput":
            shape = tuple(alloc.tensor_shape)
            dtype = mybir.dt.np(alloc.dtype)
            out_names.append(name)
            out_shapes.append((shape, dtype))
            out_avals.append(jax.core.ShapedArray(shape, dtype))
    n_params = len(in_names)
    n_outs = len(out_names)
    all_in_names = in_names + out_names
    if partition_name is not None:
        all_in_names = all_in_names + [partition_name]

    def _body(*args):
        operands = list(args)
        if partition_name is not None:
            operands.append(bass2jax.partition_id_tensor())
        outs = _bass_exec_p.bind(
            *operands, out_avals=tuple(out_avals), in_names=tuple(all_in_names),
            out_names=tuple(out_names), lowering_input_output_aliases=(),
            sim_require_finite=True, sim_require_nnan=True, nc=nc)
        return tuple(outs)

    devices = jax.devices()[:NCORES]
    mesh = Mesh(np.asarray(devices), ("core",))
    in_specs = (PartitionSpec("core"),) * (n_params + n_outs)
    out_specs = (PartitionSpec("core"),) * n_outs
    fn = jax.jit(shard_map(_body, mesh=mesh, in_specs=in_specs,
                           out_specs=out_specs, check_rep=False),
                 keep_unused=True)
    return {"fn": fn, "in_names": in_names, "out_names": out_names,
            "out_shapes": out_shapes, "nc": nc}


def _run(ex, ins):
    """ins: dict name -> [NCORES, *shape]; returns dict name -> [NCORES, *shape]."""
    concat_in = [np.ascontiguousarray(ins[n].reshape(-1, *ins[n].shape[2:]))
                 for n in ex["in_names"]]
    concat_zeros = [np.zeros((NCORES * s[0], *s[1:]), dt)
                    for (s, dt) in ex["out_shapes"]]
    out_arrs = ex["fn"](*concat_in, *concat_zeros)
    return {name: np.asarray(out_arrs[i]).reshape(NCORES, *ex["out_shapes"][i][0])
            for i, name in enumerate(ex["out_names"])}


def kernel(desc0, desc1):
    global _cached
    desc0 = np.asarray(desc0, dtype=np.float32)
    desc1 = np.asarray(desc1, dtype=np.float32)
    assert desc0.shape == (B, N, D) and desc1.shape == (B, M, D)

    if _cached is None:
        _cached = (_make_exec(_build1()), _make_exec(_build2()))
    ex1, ex2 = _cached

    a_slab = np.stack([desc0[b, h * HALF:(h + 1) * HALF]
                       for b in range(B) for h in range(2)]) \
               .astype(np.float16)                                # [8,4096,128]
    bt_all = np.stack([desc1[b].transpose(1, 0)
                       for b in range(B) for h in range(2)]) \
               .astype(np.float16)                                # [8,128,8192]
    at_all = a_slab.transpose(0, 2, 1)                            # [8,128,4096]

    r1 = _run(ex1, {"at": at_all, "bt": bt_all})

    # host glue: score/chunk-argmax + grouping for phase 2 (all fp16-exact)
    cm8 = r1["cm"].reshape(NCORES, 128, NT, NCHUNK, 64)
    cm = np.empty((NCORES, 128, NT, NCHUNK), np.float32)
    for core in range(NCORES):
        cm[core] = cm8[core].astype(np.float32).max(axis=3)
    cm = cm.transpose(0, 2, 1, 3).reshape(NCORES, HALF, NCHUNK)
    score0_c = cm.max(axis=2)                                     # [8,4096] f16
    cstar_c = cm.argmax(axis=2)                                   # [8, 4096]

    at2 = np.zeros((NCORES, D, NSLOT), np.float16)
    sg = np.full((NCORES, 128, NST * 8), 60000.0, np.float16)
    slot_of_row = np.full((NCORES, HALF), -1, np.int64)
    overflow = []                                                 # (core, row)
    for core in range(NCORES):
        for g in range(NCHUNK):
            rows_g = np.nonzero(cstar_c[core] == g)[0]
            if len(rows_g) > GCAP:
                overflow.extend((core, r) for r in rows_g[GCAP:])
                rows_g = rows_g[:GCAP]
            slots = g * GCAP + np.arange(len(rows_g))
            slot_of_row[core, rows_g] = slots
            at2[core][:, slots] = a_slab[core][rows_g].T
            st, lane = slots // 128, slots % 128
            for k in range(8):
                sg[core][lane, st * 8 + k] = score0_c[core][rows_g]

    r2 = _run(ex2, {"at2": at2, "bt": bt_all, "sg": sg})
    within = r2["idx"][:, :, ::8]                                 # [8, 128, NST]

    match01 = np.empty((B, N), dtype=np.int32)
    score0 = np.empty((B, N), dtype=np.float32)
    valid = np.empty((B, N), dtype=bool)
    # 128->1 partition reduction of the per-core column maxima on the host
    colmax = r1["colp"].astype(np.float32) \
                       .reshape(B, 2 * 128, M).max(axis=1)        # [B, M]

    for core in range(NCORES):
        b, h = divmod(core, 2)
        s = score0_c[core]
        sl = slot_of_row[core]
        m = cstar_c[core] * CHW + \
            within[core][sl % 128, sl // 128].astype(np.int64)
        sel = slice(h * HALF, (h + 1) * HALF)
        score0[b, sel] = s.astype(np.float32)
        match01[b, sel] = m.astype(np.int32)
        valid[b, sel] = (s > 0.1) & (s == colmax[b][m])

    for core, row in overflow:                                    # ~never taken
        b, h = divmod(core, 2)
        simrow = a_slab[core][row].astype(np.float32) @ desc1[b].T
        n = h * HALF + row
        match01[b, n] = int(simrow.argmax())
        score0[b, n] = simrow.max()
        valid[b, n] = (score0[b, n] > 0.1) & \
                      (np.float16(score0[b, n]) == colmax[b][match01[b, n]])

    return match01, score0, valid
